# revision 11
# baseline (speedup 1.0000x reference)
"""Trainium2 Bass kernel for a 2-layer GAT with auxiliary heads.

Distribution (8 NeuronCores, SPMD single program):
  - Destination nodes are packed into blocks of 128, balanced by in-degree.
    Core k owns blocks [k*BPC, (k+1)*BPC).
  - Every core computes the full node feature table hx = [h|alpha_s|alpha_d]
    (replicated compute beats all-gathering the 65MB table).
  - Edge aggregation per destination block: gather 128-row source-feature
    tiles with dma_gather (int16 indices -> lo/hi split of the table: lo
    reads base 0, hi reads base NS-32768; each instruction <= 1024 rows);
    build the one-hot selection matrix S[e,d] by comparing the dst-row id
    against an iota row; per-edge alpha_dst comes from a PE-transpose of S
    and a small matmul against the block's alpha_d rows (fetched with a
    single-index indirect DMA).  The segment-softmax numerator/denominator
    accumulate as TensorEngine matmuls psum[d,:] += S^T @ [h*exp | exp].
  - One small AllGather moves the per-core [h2|as2|ad2] shards between
    layers; layer 2 repeats the edge pass; 4 output heads per block.
"""

import os
import sys
import numpy as np

for _p in ("/opt/trn_rl_repo", "/root/.axon_site/_ro/trn_rl_repo"):
    if os.path.isdir(_p) and _p not in sys.path:
        sys.path.insert(0, _p)

from contextlib import ExitStack

import concourse.bass as bass
import concourse.tile as tile
from concourse import bacc, mybir
from concourse.masks import make_identity

F32 = mybir.dt.float32
I32 = mybir.dt.int32
I16 = mybir.dt.int16
AF = mybir.ActivationFunctionType
OP = mybir.AluOpType

NEG_SLOPE = 0.2
BN_EPS = 1e-5
PAD_DSTL = 30000.0
GCHUNK = 8          # max gather tiles (of 128 rows) per dma_gather


class Cfg:
    def __init__(self, n_cores, bpc, tlo, thi, n_nodes, lo_cap=None):
        self.n_cores = n_cores
        self.bpc = bpc
        self.tlo = tlo              # lo-src edge tiles per block
        self.thi = thi              # hi-src edge tiles per block
        self.te = tlo + thi         # edge tiles per block
        self.n_nodes = n_nodes
        self.in_c = 128
        self.hid = 32
        self.heads = 8
        self.hw = 256
        self.row1 = 320             # [h(256)|as(8)|ad(8)|pad] f32, 1280B
        self.r1u = 272              # used part of row1
        self.row2 = 64              # [h2(32)|as2|ad2|pad] f32, 256B
        self.r2u = 34
        self.n_slots = n_cores * bpc * 128
        self.cps = bpc * 128
        self.out_c = 82
        self.gp = 5 if bpc % 5 == 0 else 1
        self.lo_cap = min(self.n_slots, 32768) if lo_cap is None else lo_cap
        self.hi_base = max(0, self.n_slots - self.lo_cap)
        assert self.n_slots - self.hi_base <= 32768
        assert self.n_slots >= n_nodes


def _view(ap, extra_offset, dims):
    return bass.AP(ap.tensor, ap.offset + extra_offset, [ap.ap[0]] + dims)


def build_program(cfg: Cfg, taps: bool = False):
    nc = bacc.Bacc(
        "TRN2", target_bir_lowering=False, debug=False, num_devices=cfg.n_cores
    )

    NS, CPS, BPC, GP = cfg.n_slots, cfg.cps, cfg.bpc, cfg.gp
    TLO, THI, TE = cfg.tlo, cfg.thi, cfg.te
    R1, R1U, R2, R2U = cfg.row1, cfg.r1u, cfg.row2, cfg.r2u
    HW, HEADS, HID = cfg.hw, cfg.heads, cfg.hid
    R2M = HID + 1
    NT = NS // 128
    OC = cfg.out_c
    NLO, NHI = TLO * 128, THI * 128

    # gather chunking: lo tiles [0,TLO) base 0; hi tiles [TLO,TE) base hi_base
    lo_chunks = [(s, min(s + GCHUNK, TLO)) for s in range(0, TLO, GCHUNK)]
    hi_chunks = [(s, min(s + GCHUNK, TE)) for s in range(TLO, TE, GCHUNK)]

    # ---- I/O ----
    xT = nc.dram_tensor("xT", [128, NS], F32, kind="ExternalInput")
    w1ext = nc.dram_tensor("w1ext", [128, R1U], F32, kind="ExternalInput")
    w2ext = nc.dram_tensor("w2ext", [128, 2 * R2U], F32, kind="ExternalInput")
    wheads = nc.dram_tensor("wheads", [HID + 1, OC], F32, kind="ExternalInput")
    negw2sum = nc.dram_tensor("negw2sum", [1, R2U], F32, kind="ExternalInput")
    bns = nc.dram_tensor("bns", [128, HW], F32, kind="ExternalInput")
    bnt = nc.dram_tensor("bnt", [128, HW], F32, kind="ExternalInput")
    ilo = nc.dram_tensor("ilo", [128, BPC * (NLO // 16)], I16, kind="ExternalInput")
    if THI:
        ihi = nc.dram_tensor("ihi", [128, BPC * (NHI // 16)], I16,
                             kind="ExternalInput")
    adix = nc.dram_tensor("adix", [128, BPC], I32, kind="ExternalInput")
    edstl = nc.dram_tensor("edstl", [128, BPC * TE], F32, kind="ExternalInput")
    out_all = nc.dram_tensor("out_all", [CPS, OC], F32, kind="ExternalOutput")

    # ---- internal DRAM ----
    tap_kind = "ExternalOutput" if taps else "Internal"
    hx = nc.dram_tensor("hx", [NS, R1], F32, kind=tap_kind)
    hx2loc = nc.dram_tensor("hx2loc", [CPS, R2], F32, kind="Internal")
    hx2full = nc.dram_tensor("hx2full", [NS, R2], F32, kind="Internal")
    if taps:
        hx2l_out = nc.dram_tensor("hx2l_out", [CPS, R2], F32,
                                  kind="ExternalOutput")
        hx2f_out = nc.dram_tensor("hx2f_out", [NS, R2], F32,
                                  kind="ExternalOutput")

    with tile.TileContext(nc) as tc, ExitStack() as ctx:
        consts = ctx.enter_context(tc.tile_pool(name="consts", bufs=1))
        persist = ctx.enter_context(tc.tile_pool(name="persist", bufs=1))
        xload = ctx.enter_context(tc.tile_pool(name="xload", bufs=2))
        acopy = ctx.enter_context(tc.tile_pool(name="acopy", bufs=2))
        gpool = ctx.enter_context(tc.tile_pool(name="gpool", bufs=2))
        adpool = ctx.enter_context(tc.tile_pool(name="adpool", bufs=2))
        msgpool = ctx.enter_context(tc.tile_pool(name="msgpool", bufs=2))
        spool = ctx.enter_context(tc.tile_pool(name="spool", bufs=cfg.te + 2))
        sdpool = ctx.enter_context(tc.tile_pool(name="sdpool", bufs=2))
        grpool = ctx.enter_context(tc.tile_pool(name="grpool", bufs=2))
        small = ctx.enter_context(tc.tile_pool(name="small", bufs=4))
        scratch = ctx.enter_context(tc.tile_pool(name="scratch", bufs=2))
        psum = ctx.enter_context(tc.tile_pool(name="psum", bufs=4, space="PSUM"))
        pstr = ctx.enter_context(tc.tile_pool(name="pstr", bufs=2, space="PSUM"))

        # ---- constants ----
        ident = consts.tile([128, 128], F32)
        make_identity(nc, ident[:])
        r_i32 = consts.tile([128, 128], I32)
        nc.gpsimd.iota(r_i32[:], pattern=[[1, 128]], channel_multiplier=0)
        r_f32 = consts.tile([128, 128], F32)
        nc.vector.tensor_copy(r_f32[:], r_i32[:])

        w1e_sb = consts.tile([128, R1U], F32)
        nc.sync.dma_start(w1e_sb[:], w1ext[:])
        w2e_sb = consts.tile([128, 2 * R2U], F32)
        nc.sync.dma_start(w2e_sb[:], w2ext[:])
        wh_sb = consts.tile([HID + 1, OC], F32)
        nc.sync.dma_start(wh_sb[:], wheads[:])
        nw2s_sb = consts.tile([1, R2U], F32)
        nc.sync.dma_start(nw2s_sb[:], negw2sum[:])
        ones1 = consts.tile([1, 128], F32)
        nc.vector.memset(ones1[:], 1.0)
        bns_sb = consts.tile([128, HW], F32)
        nc.sync.dma_start(bns_sb[:], bns[:])
        bnt_sb = consts.tile([128, HW], F32)
        nc.sync.dma_start(bnt_sb[:], bnt[:])

        ilo_sb = persist.tile([128, BPC * (NLO // 16)], I16)
        nc.sync.dma_start(ilo_sb[:], ilo[:])
        if THI:
            ihi_sb = persist.tile([128, BPC * (NHI // 16)], I16)
            nc.sync.dma_start(ihi_sb[:], ihi[:])
        adix_sb = persist.tile([128, BPC], I32)
        nc.sync.dma_start(adix_sb[:], adix[:])
        edstl_sb = persist.tile([128, BPC * TE], F32)
        nc.sync.dma_start(edstl_sb[:], edstl[:])

        # ---- Phase A: hx[slot, 0:272] = x[slot] @ [W1|W1@As|W1@Ad] ----
        XL = 4 if NT % 4 == 0 else (2 if NT % 2 == 0 else 1)
        for nbg in range(NT // XL):
            xt = xload.tile([128, 128 * XL], F32, tag="xt")
            nc.sync.dma_start(xt[:], xT[:, nbg * 128 * XL:(nbg + 1) * 128 * XL])
            for j in range(XL):
                nb = nbg * XL + j
                ps = psum.tile([128, R1U], F32, tag="ps")
                nc.tensor.matmul(
                    ps[:], lhsT=xt[:, j * 128:(j + 1) * 128],
                    rhs=w1e_sb[:], start=True, stop=True,
                )
                hxt = acopy.tile([128, R1U], F32, tag="hxt")
                if nb % 2 == 0:
                    nc.vector.tensor_copy(hxt[:], ps[:])
                else:
                    nc.scalar.copy(hxt[:], ps[:])
                nc.sync.dma_start(hx[nb * 128:(nb + 1) * 128, 0:R1U], hxt[:])

        def edge_pass(layer, table, row, rowu, msg_w, nheads, ad_col, get_adblk,
                      post_block):
            """Shared edge-aggregation pass. msg row = [msg_w*nheads | nheads]."""
            rtile = row * 128 // 128  # elements per gathered row
            mrow = msg_w * nheads + nheads
            for b in range(BPC):
                hx_g = gpool.tile([128, TE * row], F32, tag=f"g{layer}")
                for (s, e) in lo_chunks:
                    nt = e - s
                    gv = bass.AP(hx_g.tensor, hx_g.offset + s * row,
                                 [hx_g.ap[0], [row, nt], [1, row]])
                    nc.gpsimd.dma_gather(
                        out_ap=gv, in_ap=table[:],
                        idxs_ap=ilo_sb[:, b * (NLO // 16) + s * 8:
                                       b * (NLO // 16) + e * 8],
                        num_idxs=nt * 128, num_idxs_reg=nt * 128,
                        elem_size=row,
                    )
                for (s, e) in hi_chunks:
                    nt = e - s
                    gv = bass.AP(hx_g.tensor, hx_g.offset + s * row,
                                 [hx_g.ap[0], [row, nt], [1, row]])
                    nc.gpsimd.dma_gather(
                        out_ap=gv, in_ap=table[cfg.hi_base:, :],
                        idxs_ap=ihi_sb[:, b * (NHI // 16) + (s - TLO) * 8:
                                       b * (NHI // 16) + (e - TLO) * 8],
                        num_idxs=nt * 128, num_idxs_reg=nt * 128,
                        elem_size=row,
                    )
                ad_blk = get_adblk(b)

                # selection matrices + alpha_dst expansion
                ad_all = pstr.tile([128, TE * nheads], F32, tag="adall")
                seds = []
                for t0 in range(0, TE, 4):
                    t1 = min(t0 + 4, TE)
                    ptr = pstr.tile([128, 512], F32, tag="tr")
                    for t in range(t0, t1):
                        sed = spool.tile([128, 128], F32, tag="sed")
                        dcol = edstl_sb[:, b * TE + t:b * TE + t + 1]
                        nc.vector.tensor_tensor(
                            out=sed[:], in0=dcol.to_broadcast([128, 128]),
                            in1=r_f32[:], op=OP.is_equal,
                        )
                        seds.append(sed)
                        nc.tensor.transpose(
                            ptr[:, (t - t0) * 128:(t - t0 + 1) * 128],
                            sed[:], ident[:],
                        )
                    sde = sdpool.tile([128, 512], F32, tag="sde")
                    nc.scalar.copy(sde[:, 0:(t1 - t0) * 128],
                                   ptr[:, 0:(t1 - t0) * 128])
                    for t in range(t0, t1):
                        nc.tensor.matmul(
                            ad_all[:, t * nheads:(t + 1) * nheads],
                            lhsT=sde[:, (t - t0) * 128:(t - t0 + 1) * 128],
                            rhs=ad_blk, start=True, stop=True,
                        )

                # logits -> leaky relu -> exp -> weighted messages (batched)
                as_view = _view(hx_g, msg_w * nheads,
                                [[row, TE], [1, nheads]])
                lg = small.tile([128, TE * nheads], F32, tag=f"lg{layer}")
                nc.vector.tensor_tensor(out=lg[:], in0=as_view, in1=ad_all[:],
                                        op=OP.add)
                nc.vector.scalar_tensor_tensor(
                    out=lg[:], in0=lg[:], scalar=NEG_SLOPE, in1=lg[:],
                    op0=OP.mult, op1=OP.max,
                )
                msg = msgpool.tile([128, TE * mrow], F32, tag=f"m{layer}")
                el_dst = _view(msg, msg_w * nheads, [[mrow, TE], [1, nheads]])
                nc.scalar.activation(el_dst, lg[:], AF.Exp)
                h_view = _view(hx_g, 0, [[row, TE], [msg_w, nheads], [1, msg_w]])
                el_exp = _view(msg, msg_w * nheads,
                               [[mrow, TE], [1, nheads], [0, msg_w]])
                msg_dst = _view(msg, 0, [[mrow, TE], [msg_w, nheads], [1, msg_w]])
                nc.vector.tensor_tensor(out=msg_dst, in0=h_view, in1=el_exp,
                                        op=OP.mult)

                ps_agg = psum.tile([128, mrow], F32, tag="ps")
                for t in range(TE):
                    nc.tensor.matmul(
                        ps_agg[:], lhsT=seds[t][:],
                        rhs=msg[:, t * mrow:(t + 1) * mrow],
                        start=(t == 0), stop=(t == TE - 1),
                    )
                post_block(b, ps_agg)

        # ---- Layer 1 ----
        o1gs = {}

        def get_adblk1(b):
            adrow = adpool.tile([128, R1], F32, tag="adrow")
            nc.gpsimd.indirect_dma_start(
                out=adrow[:], out_offset=None, in_=hx[:],
                in_offset=bass.IndirectOffsetOnAxis(
                    ap=adix_sb[:, b:b + 1], axis=0),
            )
            return adrow[:, HW + HEADS:HW + 2 * HEADS]

        def post1(b, ps_agg):
            g = b // GP
            bi = b % GP
            if bi == 0:
                o1g_t = grpool.tile([128, GP * HW], F32, tag="o1g")
                d1g_t = grpool.tile([128, GP * HEADS], F32, tag="d1g")
                o1gs[g] = (o1g_t, d1g_t)
            o1g, d1g = o1gs[g]
            nc.vector.tensor_copy(o1g[:, bi * HW:(bi + 1) * HW],
                                  ps_agg[:, 0:HW])
            nc.vector.tensor_scalar_add(
                d1g[:, bi * HEADS:(bi + 1) * HEADS],
                ps_agg[:, HW:HW + HEADS], 1e-16)
            if bi != GP - 1:
                return
            # post: y = (num*recip)*s + t; elu_shift = elu(y)+1; then W2
            r1g = grpool.tile([128, GP * HEADS], F32, tag="r1g")
            nc.vector.reciprocal(r1g[:], d1g[:])
            o1v = _view(o1g, 0, [[HW, GP], [HID, HEADS], [1, HID]])
            rev = _view(r1g, 0, [[HEADS, GP], [1, HEADS], [0, HID]])
            nc.vector.tensor_tensor(out=o1v, in0=o1v, in1=rev, op=OP.mult)
            sv = _view(bns_sb, 0, [[0, GP], [1, HW]])
            tv = _view(bnt_sb, 0, [[0, GP], [1, HW]])
            o1f = _view(o1g, 0, [[HW, GP], [1, HW]])
            nc.vector.tensor_tensor(out=o1f, in0=o1f, in1=sv, op=OP.mult)
            nc.vector.tensor_tensor(out=o1f, in0=o1f, in1=tv, op=OP.add)
            mn = scratch.tile([128, GP * HW], F32, tag="mn")
            nc.vector.tensor_scalar_min(mn[:], o1g[:], 0.0)
            nc.scalar.activation(mn[:], mn[:], AF.Exp)
            nc.vector.scalar_tensor_tensor(
                out=o1g[:], in0=o1g[:], scalar=0.0, in1=mn[:],
                op0=OP.max, op1=OP.add,
            )
            for bj in range(GP):
                bb = g * GP + bj
                e2 = small.tile([128, 2 * 128], F32, tag="eluT")
                for j in range(2):
                    pt = pstr.tile([128, 512], F32, tag="tr")
                    nc.tensor.transpose(
                        pt[:, 0:128],
                        o1g[:, bj * HW + j * 128: bj * HW + (j + 1) * 128],
                        ident[:],
                    )
                    nc.vector.tensor_copy(e2[:, j * 128:(j + 1) * 128],
                                          pt[:, 0:128])
                ph2 = psum.tile([128, R2U], F32, tag="ps")
                for j in range(2):
                    nc.tensor.matmul(
                        ph2[:], lhsT=e2[:, j * 128:(j + 1) * 128],
                        rhs=w2e_sb[:, j * R2U:(j + 1) * R2U],
                        start=(j == 0), stop=False,
                    )
                nc.tensor.matmul(ph2[:], lhsT=ones1[:], rhs=nw2s_sb[:],
                                 start=False, stop=True)
                h2sb = small.tile([128, R2U], F32, tag="h2sb")
                nc.vector.tensor_copy(h2sb[:], ph2[:])
                nc.sync.dma_start(hx2loc[bb * 128:(bb + 1) * 128, 0:R2U],
                                  h2sb[:])

        edge_pass(1, hx, R1, R1U, HID, HEADS, HW + HEADS, get_adblk1, post1)

        # ---- AllGather layer-2 table ----
        nc.gpsimd.collective_compute(
            "AllGather", OP.bypass,
            replica_groups=[list(range(cfg.n_cores))],
            ins=[hx2loc[:].opt()],
            outs=[hx2full[:].opt()],
        )
        if taps:
            nc.sync.dma_start(hx2l_out[:], hx2loc[:])
            nc.sync.dma_start(hx2f_out[:], hx2full[:])

        # ---- Layer 2 + heads ----
        lhsT33 = persist.tile([HID + 1, 128], F32)
        nc.vector.memset(lhsT33[HID:HID + 1, :], 1.0)

        def get_adblk2(b):
            ad2 = adpool.tile([128, 1], F32, tag="ad2")
            nc.sync.dma_start(ad2[:], hx2loc[b * 128:(b + 1) * 128,
                                             HID + 1:HID + 2])
            return ad2[:]

        def post2(b, ps2):
            den2 = small.tile([128, 1], F32, tag="den2")
            nc.vector.tensor_scalar_add(den2[:], ps2[:, HID:HID + 1], 1e-16)
            rec2 = small.tile([128, 1], F32, tag="rec2")
            nc.vector.reciprocal(rec2[:], den2[:])
            out2 = small.tile([128, HID], F32, tag="out2")
            nc.vector.tensor_scalar(
                out=out2[:], in0=ps2[:, 0:HID], scalar1=rec2[:],
                scalar2=None, op0=OP.mult,
            )
            pt = psum.tile([HID, 128], F32, tag="ps")
            nc.tensor.transpose(pt[:], out2[:], ident[:])
            nc.vector.tensor_copy(lhsT33[0:HID, :], pt[:])
            pz = psum.tile([128, OC], F32, tag="ps")
            nc.tensor.matmul(pz[:], lhsT=lhsT33[:], rhs=wh_sb[:],
                             start=True, stop=True)
            osb = scratch.tile([128, OC], F32, tag="osb")
            e1 = scratch.tile([128, 40], F32, tag="e1")
            se1 = small.tile([128, 1], F32, tag="se")
            nc.scalar.activation(e1[:], pz[:, 0:40], AF.Exp, accum_out=se1[:])
            lse = small.tile([128, 1], F32, tag="lse")
            nc.scalar.activation(lse[:], se1[:], AF.Ln)
            nc.vector.tensor_scalar(
                out=osb[:, 0:40], in0=pz[:, 0:40], scalar1=lse[:],
                scalar2=None, op0=OP.subtract,
            )
            e2h = scratch.tile([128, 40], F32, tag="e2h")
            se2 = small.tile([128, 1], F32, tag="se")
            nc.scalar.activation(e2h[:], pz[:, 40:80], AF.Exp, accum_out=se2[:])
            re2 = small.tile([128, 1], F32, tag="re2")
            nc.vector.reciprocal(re2[:], se2[:])
            nc.vector.tensor_scalar(
                out=osb[:, 40:80], in0=e2h[:], scalar1=re2[:],
                scalar2=None, op0=OP.mult,
            )
            e3 = small.tile([128, 2], F32, tag="e3")
            nc.scalar.activation(e3[:], pz[:, 80:82], AF.Exp, scale=-1.0)
            nc.vector.tensor_scalar_add(e3[:], e3[:], 1.0)
            nc.vector.reciprocal(osb[:, 80:82], e3[:])
            nc.sync.dma_start(out_all[b * 128:(b + 1) * 128, :], osb[:])

        edge_pass(2, hx2full, R2, R2U, HID, 1, HID, get_adblk2, post2)

    nc.compile()
    return nc


# ----------------------------------------------------------------------------
# Host-side graph partitioning / input preparation
# ----------------------------------------------------------------------------
def wrap16(idx_2d):
    """[nblk, n] int -> [128, nblk*(n/16)] int16 (16-wrap, replicated 8x)."""
    nblk, n = idx_2d.shape
    a = idx_2d.reshape(nblk, n // 16, 16).transpose(2, 0, 1).reshape(
        16, nblk * (n // 16))
    return np.ascontiguousarray(np.tile(a.astype(np.int16), (8, 1)))


def partition_graph(edge_index, n_nodes, n_cores, bpc, lo_cap, tlo, thi):
    src = np.concatenate([np.asarray(edge_index[0], dtype=np.int64),
                          np.arange(n_nodes, dtype=np.int64)])
    dst = np.concatenate([np.asarray(edge_index[1], dtype=np.int64),
                          np.arange(n_nodes, dtype=np.int64)])
    n_blocks = n_cores * bpc
    n_slots = n_blocks * 128
    hi_base = max(0, n_slots - lo_cap)
    deg = np.bincount(dst, minlength=n_nodes).astype(np.int64)

    order = np.argsort(-deg, kind="stable")
    loads = np.zeros(n_blocks, dtype=np.int64)
    blk_of_node = np.empty(n_nodes, dtype=np.int64)
    pos = 0
    while pos < n_nodes:
        take = min(n_blocks, n_nodes - pos)
        chunk = order[pos:pos + take]
        light = np.argsort(loads, kind="stable")[:take]
        blk_of_node[chunk] = light
        loads[light] += deg[chunk]
        pos += take

    row_of_node = np.empty(n_nodes, dtype=np.int64)
    blk_sorted = np.argsort(blk_of_node, kind="stable")
    bc = np.bincount(blk_of_node, minlength=n_blocks)
    assert bc.max() <= 128
    starts = np.concatenate([[0], np.cumsum(bc)[:-1]])
    row_of_node[blk_sorted] = np.arange(n_nodes) - starts[blk_of_node[blk_sorted]]
    slot_of_node = blk_of_node * 128 + row_of_node

    eblk = blk_of_node[dst].astype(np.int64)
    esrc_slot = slot_of_node[src]
    edst_row = row_of_node[dst].astype(np.float32)
    is_lo = esrc_slot < lo_cap

    NLO, NHI = tlo * 128, thi * 128
    lo_idx = np.zeros((n_blocks, NLO), dtype=np.int64)
    hi_idx = np.zeros((n_blocks, max(NHI, 1)), dtype=np.int64)
    dstl = np.full((n_blocks, (tlo + thi) * 128), PAD_DSTL, dtype=np.float32)

    # place edges: per block, lo edges then hi edges
    for side, sel in ((0, is_lo), (1, ~is_lo)):
        eb = eblk[sel]
        es = esrc_slot[sel]
        er = edst_row[sel]
        order_e = np.argsort(eb, kind="stable")
        eb, es, er = eb[order_e], es[order_e], er[order_e]
        cnt = np.bincount(eb, minlength=n_blocks)
        st = np.concatenate([[0], np.cumsum(cnt)[:-1]])
        pos_in = np.arange(len(eb)) - st[eb]
        if side == 0:
            assert cnt.max() <= NLO, f"lo overflow {cnt.max()}>{NLO}"
            lo_idx[eb, pos_in] = es
            dstl[eb, pos_in] = er
        else:
            assert cnt.max() <= max(NHI, 1), f"hi overflow {cnt.max()}>{NHI}"
            if NHI:
                hi_idx[eb, pos_in] = es - hi_base
                dstl[eb, NLO + pos_in] = er

    per_core = []
    for k in range(n_cores):
        blo = lo_idx[k * bpc:(k + 1) * bpc]
        pc = {"ilo": wrap16(blo)}
        if NHI:
            pc["ihi"] = wrap16(hi_idx[k * bpc:(k + 1) * bpc])
        gb = (np.arange(bpc) + k * bpc) * 128
        pc["adix"] = np.ascontiguousarray(
            (gb[None, :] + np.arange(128)[:, None]).astype(np.int32))
        d = dstl[k * bpc:(k + 1) * bpc].reshape(bpc, tlo + thi, 128)
        pc["edstl"] = np.ascontiguousarray(
            d.transpose(2, 0, 1).reshape(128, bpc * (tlo + thi)))
        per_core.append(pc)
    return slot_of_node, per_core


def fold_weights(inp, cfg):
    HID, HEADS, HW = cfg.hid, cfg.heads, cfg.hw
    g = lambda k: np.asarray(inp[k], dtype=np.float64)
    W1, a_src1, a_dst1 = g("W1"), g("a_src1"), g("a_dst1")
    As1 = np.zeros((HW, HEADS))
    Ad1 = np.zeros((HW, HEADS))
    for h in range(HEADS):
        As1[h * HID:(h + 1) * HID, h] = a_src1[h]
        Ad1[h * HID:(h + 1) * HID, h] = a_dst1[h]
    w1ext = np.concatenate([W1, W1 @ As1, W1 @ Ad1], axis=1).astype(np.float32)

    s = g("bn_gamma") / np.sqrt(g("bn_var") + BN_EPS)
    t = (g("b1") - g("bn_mean")) * s + g("bn_beta")
    bns = np.tile(s.astype(np.float32), (128, 1))
    bnt = np.tile(t.astype(np.float32), (128, 1))

    W2 = g("W2")
    w2e = np.concatenate(
        [W2, W2 @ g("a_src2").reshape(-1, 1), W2 @ g("a_dst2").reshape(-1, 1)],
        axis=1)
    negw2sum = (-w2e.sum(axis=0)).astype(np.float32)[None, :]
    w2ext = np.concatenate([w2e[0:128], w2e[128:256]], axis=1).astype(np.float32)

    Wall = np.concatenate([g("Wc"), g("Ws"), g("Wh"), g("We")], axis=1)
    ball = np.concatenate([g("bc"), g("bs"), g("bh"), g("be")])
    ball = ball + g("b2") @ Wall
    wheads = np.concatenate([Wall, ball[None, :]], axis=0).astype(np.float32)
    return w1ext, bns, bnt, w2ext, negw2sum, wheads


def prepare_inputs(inputs, n_cores, bpc, tlo, thi, lo_cap=None):
    x = np.asarray(inputs["x"], dtype=np.float32)
    edge_index = np.asarray(inputs["edge_index"])
    n = x.shape[0]
    cfg = Cfg(n_cores=n_cores, bpc=bpc, tlo=tlo, thi=thi, n_nodes=n,
              lo_cap=lo_cap)
    slot_of_node, per_core = partition_graph(
        edge_index, n, n_cores, bpc, cfg.lo_cap, tlo, thi)

    asg2orig = np.zeros(cfg.n_slots, dtype=np.int64)
    asg2orig[slot_of_node] = np.arange(n)
    xT = np.ascontiguousarray(x[asg2orig].T)

    w1ext, bns, bnt, w2ext, negw2sum, wheads = fold_weights(inputs, cfg)
    in_maps = []
    for k in range(n_cores):
        m = {"xT": xT, "w1ext": w1ext, "w2ext": w2ext, "wheads": wheads,
             "negw2sum": negw2sum, "bns": bns, "bnt": bnt}
        m.update(per_core[k])
        in_maps.append(m)
    return cfg, in_maps, slot_of_node


def assemble_outputs(results, slot_of_node, cfg: Cfg):
    full = np.concatenate(
        [np.asarray(results[k]["out_all"]) for k in range(cfg.n_cores)], axis=0)
    rows = full[slot_of_node]
    return (np.ascontiguousarray(rows[:, 0:40]),
            np.ascontiguousarray(rows[:, 40:80]),
            np.ascontiguousarray(rows[:, 80]),
            np.ascontiguousarray(rows[:, 81]))


# ----------------------------------------------------------------------------
# Entry point
# ----------------------------------------------------------------------------
_compiled = {}


def _get_program(cfg: Cfg):
    key = (cfg.n_cores, cfg.bpc, cfg.tlo, cfg.thi, cfg.n_nodes, cfg.lo_cap)
    if key not in _compiled:
        _compiled[key] = build_program(cfg)
    return _compiled[key]


def kernel(**inputs):
    from concourse import bass_utils

    cfg, in_maps, slot_of_node = prepare_inputs(
        inputs, n_cores=8, bpc=50, tlo=12, thi=7)
    nc = _get_program(cfg)
    res = bass_utils.run_bass_kernel_spmd(
        nc, in_maps, core_ids=list(range(cfg.n_cores)))
    return assemble_outputs(res.results, slot_of_node, cfg)


# revision 14
# speedup vs baseline: 1.5937x; 1.5937x over previous
"""Trainium2 Bass kernel for a 2-layer GAT with auxiliary heads.

Distribution (8 NeuronCores, SPMD single program):
  - Destination nodes are packed into blocks of 128, balanced by in-degree.
    Core k owns blocks [k*BPC, (k+1)*BPC).
  - Every core computes the full node feature table hx = [h|alpha_s|alpha_d]
    (replicated compute beats all-gathering the 65MB table).
  - Edge aggregation per destination block: gather 128-row source-feature
    tiles with dma_gather (int16 indices -> lo/hi split of the table: lo
    reads base 0, hi reads base NS-32768; each instruction <= 1024 rows);
    build the one-hot selection matrix S[e,d] by comparing the dst-row id
    against an iota row; per-edge alpha_dst comes from a PE-transpose of S
    and a small matmul against the block's alpha_d rows (fetched with a
    single-index indirect DMA).  The segment-softmax numerator/denominator
    accumulate as TensorEngine matmuls psum[d,:] += S^T @ [h*exp | exp].
  - One small AllGather moves the per-core [h2|as2|ad2] shards between
    layers; layer 2 repeats the edge pass; 4 output heads per block.
"""

import os
import sys
import numpy as np
import ml_dtypes

for _p in ("/opt/trn_rl_repo", "/root/.axon_site/_ro/trn_rl_repo"):
    if os.path.isdir(_p) and _p not in sys.path:
        sys.path.insert(0, _p)

from contextlib import ExitStack

import concourse.bass as bass
import concourse.tile as tile
from concourse import bacc, mybir
from concourse.masks import make_identity

F32 = mybir.dt.float32
BF = mybir.dt.bfloat16
I32 = mybir.dt.int32
I16 = mybir.dt.int16
AF = mybir.ActivationFunctionType
OP = mybir.AluOpType

NEG_SLOPE = 0.2
BN_EPS = 1e-5
PAD_DSTL = 30000.0
GCHUNK = 8          # max gather tiles (of 128 rows) per dma_gather


class Cfg:
    def __init__(self, n_cores, bpc, tlo, thi, n_nodes, lo_cap=None):
        self.n_cores = n_cores
        self.bpc = bpc
        self.tlo = tlo              # lo-src edge tiles per block
        self.thi = thi              # hi-src edge tiles per block
        self.te = tlo + thi         # edge tiles per block
        self.n_nodes = n_nodes
        self.in_c = 128
        self.hid = 32
        self.heads = 8
        self.hw = 256
        self.row1 = 384             # bf16 row: [h bf16(256)|as f32(8)|ad f32(8)|pad] = 768B
        self.r1u = 272              # f32 elements computed in phase A
        self.row2 = 128             # bf16 row: [h2 bf16(32)|as2 f32|ad2 f32|pad] = 256B
        self.r2u = 34
        self.n_slots = n_cores * bpc * 128
        self.cps = bpc * 128
        self.out_c = 82
        self.gp = 5 if bpc % 5 == 0 else 1
        self.lo_cap = min(self.n_slots, 32768) if lo_cap is None else lo_cap
        self.hi_base = max(0, self.n_slots - self.lo_cap)
        assert self.n_slots - self.hi_base <= 32768
        assert self.n_slots >= n_nodes


def _view(ap, extra_offset, dims):
    return bass.AP(ap.tensor, ap.offset + extra_offset, [ap.ap[0]] + dims)


def build_program(cfg: Cfg, taps: bool = False):
    nc = bacc.Bacc(
        "TRN2", target_bir_lowering=False, debug=False, num_devices=cfg.n_cores
    )

    NS, CPS, BPC, GP = cfg.n_slots, cfg.cps, cfg.bpc, cfg.gp
    TLO, THI, TE = cfg.tlo, cfg.thi, cfg.te
    R1, R1U, R2, R2U = cfg.row1, cfg.r1u, cfg.row2, cfg.r2u
    HW, HEADS, HID = cfg.hw, cfg.heads, cfg.hid
    R2M = HID + 1
    NT = NS // 128
    OC = cfg.out_c
    NLO, NHI = TLO * 128, THI * 128

    # gather chunking: lo tiles [0,TLO) base 0; hi tiles [TLO,TE) base hi_base
    lo_chunks = [(s, min(s + GCHUNK, TLO)) for s in range(0, TLO, GCHUNK)]
    hi_chunks = [(s, min(s + GCHUNK, TE)) for s in range(TLO, TE, GCHUNK)]

    # ---- I/O ----
    xT = nc.dram_tensor("xT", [128, NS], F32, kind="ExternalInput")
    w1ext = nc.dram_tensor("w1ext", [128, R1U], F32, kind="ExternalInput")
    w2ext = nc.dram_tensor("w2ext", [128, 2 * R2U], F32, kind="ExternalInput")
    wheads = nc.dram_tensor("wheads", [HID + 1, OC], F32, kind="ExternalInput")
    negw2sum = nc.dram_tensor("negw2sum", [1, R2U], F32, kind="ExternalInput")
    bns = nc.dram_tensor("bns", [128, HW], F32, kind="ExternalInput")
    bnt = nc.dram_tensor("bnt", [128, HW], F32, kind="ExternalInput")
    ilo = nc.dram_tensor("ilo", [128, BPC * (NLO // 16)], I16, kind="ExternalInput")
    if THI:
        ihi = nc.dram_tensor("ihi", [128, BPC * (NHI // 16)], I16,
                             kind="ExternalInput")
    adix = nc.dram_tensor("adix", [128, BPC], I32, kind="ExternalInput")
    edstl = nc.dram_tensor("edstl", [128, BPC * TE], BF, kind="ExternalInput")
    out_all = nc.dram_tensor("out_all", [CPS, OC], F32, kind="ExternalOutput")

    # ---- internal DRAM ----
    tap_kind = "ExternalOutput" if taps else "Internal"
    hx = nc.dram_tensor("hx", [NS, R1], BF, kind=tap_kind)
    hx2loc = nc.dram_tensor("hx2loc", [CPS, R2], BF, kind="Internal")
    hx2full = nc.dram_tensor("hx2full", [NS, R2], BF, kind="Internal")

    with tile.TileContext(nc) as tc, ExitStack() as ctx:
        consts = ctx.enter_context(tc.tile_pool(name="consts", bufs=1))
        persist = ctx.enter_context(tc.tile_pool(name="persist", bufs=1))
        xload = ctx.enter_context(tc.tile_pool(name="xload", bufs=2))
        acopy = ctx.enter_context(tc.tile_pool(name="acopy", bufs=2))
        gpool = ctx.enter_context(tc.tile_pool(name="gpool", bufs=2))
        adpool = ctx.enter_context(tc.tile_pool(name="adpool", bufs=2))
        msgpool = ctx.enter_context(tc.tile_pool(name="msgpool", bufs=2))
        spool = ctx.enter_context(tc.tile_pool(name="spool", bufs=cfg.te + 2))
        sdpool = ctx.enter_context(tc.tile_pool(name="sdpool", bufs=2))
        grpool = ctx.enter_context(tc.tile_pool(name="grpool", bufs=2))
        small = ctx.enter_context(tc.tile_pool(name="small", bufs=4))
        scratch = ctx.enter_context(tc.tile_pool(name="scratch", bufs=2))
        psum = ctx.enter_context(tc.tile_pool(name="psum", bufs=4, space="PSUM"))
        pstr = ctx.enter_context(tc.tile_pool(name="pstr", bufs=2, space="PSUM"))

        # ---- constants ----
        ident = consts.tile([128, 128], F32)
        make_identity(nc, ident[:])
        ident_bf = consts.tile([128, 128], BF)
        make_identity(nc, ident_bf[:])
        r_i32 = consts.tile([128, 128], I32)
        nc.gpsimd.iota(r_i32[:], pattern=[[1, 128]], channel_multiplier=0)
        r_f32 = consts.tile([128, 128], BF)
        nc.vector.tensor_copy(r_f32[:], r_i32[:])

        w1e_sb = consts.tile([128, R1U], F32)
        nc.sync.dma_start(w1e_sb[:], w1ext[:])
        w2e_sb = consts.tile([128, 2 * R2U], F32)
        nc.sync.dma_start(w2e_sb[:], w2ext[:])
        wh_sb = consts.tile([HID + 1, OC], F32)
        nc.sync.dma_start(wh_sb[:], wheads[:])
        nw2s_sb = consts.tile([1, R2U], F32)
        nc.sync.dma_start(nw2s_sb[:], negw2sum[:])
        ones1 = consts.tile([1, 128], F32)
        nc.vector.memset(ones1[:], 1.0)
        bns_sb = consts.tile([128, HW], F32)
        nc.sync.dma_start(bns_sb[:], bns[:])
        bnt_sb = consts.tile([128, HW], F32)
        nc.sync.dma_start(bnt_sb[:], bnt[:])

        ilo_sb = persist.tile([128, BPC * (NLO // 16)], I16)
        nc.sync.dma_start(ilo_sb[:], ilo[:])
        if THI:
            ihi_sb = persist.tile([128, BPC * (NHI // 16)], I16)
            nc.sync.dma_start(ihi_sb[:], ihi[:])
        adix_sb = persist.tile([128, BPC], I32)
        nc.sync.dma_start(adix_sb[:], adix[:])
        edstl_sb = persist.tile([128, BPC * TE], BF)
        nc.sync.dma_start(edstl_sb[:], edstl[:])

        # ---- Phase A: hx[slot, 0:272] = x[slot] @ [W1|W1@As|W1@Ad] ----
        XL = 4 if NT % 4 == 0 else (2 if NT % 2 == 0 else 1)
        for nbg in range(NT // XL):
            xt = xload.tile([128, 128 * XL], F32, tag="xt")
            nc.sync.dma_start(xt[:], xT[:, nbg * 128 * XL:(nbg + 1) * 128 * XL])
            for j in range(XL):
                nb = nbg * XL + j
                ps = psum.tile([128, R1U], F32, tag="ps")
                nc.tensor.matmul(
                    ps[:], lhsT=xt[:, j * 128:(j + 1) * 128],
                    rhs=w1e_sb[:], start=True, stop=True,
                )
                hxb = acopy.tile([128, HW], BF, tag="hxb")
                hxa = acopy.tile([128, 16], F32, tag="hxa")
                if nb % 2 == 0:
                    nc.vector.tensor_copy(hxb[:], ps[:, 0:HW])
                    nc.scalar.copy(hxa[:], ps[:, HW:HW + 16])
                else:
                    nc.scalar.copy(hxb[:], ps[:, 0:HW])
                    nc.vector.tensor_copy(hxa[:], ps[:, HW:HW + 16])
                nc.sync.dma_start(hx[nb * 128:(nb + 1) * 128, 0:HW], hxb[:])
                nc.sync.dma_start(
                    hx[:].bitcast(F32)[nb * 128:(nb + 1) * 128, HW // 2:HW // 2 + 16],
                    hxa[:])

        def edge_pass(layer, table, row, rowu, msg_w, nheads, ad_col, get_adblk,
                      post_block):
            """Shared edge-aggregation pass. msg row = [msg_w*nheads | nheads]."""
            rtile = row * 128 // 128  # elements per gathered row
            mrow = msg_w * nheads + nheads
            for b in range(BPC):
                hx_g = gpool.tile([128, TE * row], BF, tag=f"g{layer}")
                for (s, e) in lo_chunks:
                    nt = e - s
                    gv = bass.AP(hx_g.tensor, hx_g.offset + s * row,
                                 [hx_g.ap[0], [row, nt], [1, row]])
                    nc.gpsimd.dma_gather(
                        out_ap=gv, in_ap=table[:],
                        idxs_ap=ilo_sb[:, b * (NLO // 16) + s * 8:
                                       b * (NLO // 16) + e * 8],
                        num_idxs=nt * 128, num_idxs_reg=nt * 128,
                        elem_size=row,
                    )
                for (s, e) in hi_chunks:
                    nt = e - s
                    gv = bass.AP(hx_g.tensor, hx_g.offset + s * row,
                                 [hx_g.ap[0], [row, nt], [1, row]])
                    nc.gpsimd.dma_gather(
                        out_ap=gv, in_ap=table[cfg.hi_base:, :],
                        idxs_ap=ihi_sb[:, b * (NHI // 16) + (s - TLO) * 8:
                                       b * (NHI // 16) + (e - TLO) * 8],
                        num_idxs=nt * 128, num_idxs_reg=nt * 128,
                        elem_size=row,
                    )
                ad_blk = get_adblk(b)

                # selection matrices + alpha_dst expansion
                ad_all = pstr.tile([128, TE * nheads], F32, tag="adall")
                seds = []
                for t0 in range(0, TE, 4):
                    t1 = min(t0 + 4, TE)
                    ptr = pstr.tile([128, 512], BF, tag="tr")
                    for t in range(t0, t1):
                        sed = spool.tile([128, 128], BF, tag="sed")
                        dcol = edstl_sb[:, b * TE + t:b * TE + t + 1]
                        nc.vector.tensor_tensor(
                            out=sed[:], in0=dcol.to_broadcast([128, 128]),
                            in1=r_f32[:], op=OP.is_equal,
                        )
                        seds.append(sed)
                        nc.tensor.transpose(
                            ptr[:, (t - t0) * 128:(t - t0 + 1) * 128],
                            sed[:], ident_bf[:],
                        )
                    sde = sdpool.tile([128, 512], BF, tag="sde")
                    nc.scalar.copy(sde[:, 0:(t1 - t0) * 128],
                                   ptr[:, 0:(t1 - t0) * 128])
                    for t in range(t0, t1):
                        nc.tensor.matmul(
                            ad_all[:, t * nheads:(t + 1) * nheads],
                            lhsT=sde[:, (t - t0) * 128:(t - t0 + 1) * 128],
                            rhs=ad_blk, start=True, stop=True,
                        )

                # logits -> leaky relu -> exp -> weighted messages (batched)
                hx_gf = hx_g.bitcast(F32)
                as_view = _view(hx_gf, msg_w * nheads // 2,
                                [[row // 2, TE], [1, nheads]])
                lg = small.tile([128, TE * nheads], F32, tag=f"lg{layer}")
                nc.vector.tensor_tensor(out=lg[:], in0=as_view, in1=ad_all[:],
                                        op=OP.add)
                nc.vector.scalar_tensor_tensor(
                    out=lg[:], in0=lg[:], scalar=NEG_SLOPE, in1=lg[:],
                    op0=OP.mult, op1=OP.max,
                )
                msg = msgpool.tile([128, TE * mrow], BF, tag=f"m{layer}")
                el_dst = _view(msg, msg_w * nheads, [[mrow, TE], [1, nheads]])
                nc.scalar.activation(el_dst, lg[:], AF.Exp)
                h_view = _view(hx_g, 0, [[row, TE], [msg_w, nheads], [1, msg_w]])
                el_exp = _view(msg, msg_w * nheads,
                               [[mrow, TE], [1, nheads], [0, msg_w]])
                msg_dst = _view(msg, 0, [[mrow, TE], [msg_w, nheads], [1, msg_w]])
                nc.vector.tensor_tensor(out=msg_dst, in0=h_view, in1=el_exp,
                                        op=OP.mult)

                ps_agg = psum.tile([128, mrow], F32, tag="ps")
                for t in range(TE):
                    nc.tensor.matmul(
                        ps_agg[:], lhsT=seds[t][:],
                        rhs=msg[:, t * mrow:(t + 1) * mrow],
                        start=(t == 0), stop=(t == TE - 1),
                    )
                post_block(b, ps_agg)

        # ---- Layer 1 ----
        o1gs = {}

        def get_adblk1(b):
            adrow = adpool.tile([128, R1], BF, tag="adrow")
            nc.gpsimd.indirect_dma_start(
                out=adrow[:], out_offset=None, in_=hx[:],
                in_offset=bass.IndirectOffsetOnAxis(
                    ap=adix_sb[:, b:b + 1], axis=0),
            )
            adb = adpool.tile([128, HEADS], BF, tag="adb")
            nc.vector.tensor_copy(
                adb[:], adrow.bitcast(F32)[:, HW // 2 + HEADS:HW // 2 + 2 * HEADS])
            return adb[:]

        def post1(b, ps_agg):
            g = b // GP
            bi = b % GP
            if bi == 0:
                o1g_t = grpool.tile([128, GP * HW], F32, tag="o1g")
                d1g_t = grpool.tile([128, GP * HEADS], F32, tag="d1g")
                o1gs[g] = (o1g_t, d1g_t)
            o1g, d1g = o1gs[g]
            nc.vector.tensor_copy(o1g[:, bi * HW:(bi + 1) * HW],
                                  ps_agg[:, 0:HW])
            nc.vector.tensor_scalar_add(
                d1g[:, bi * HEADS:(bi + 1) * HEADS],
                ps_agg[:, HW:HW + HEADS], 1e-16)
            if bi != GP - 1:
                return
            # post: y = (num*recip)*s + t; elu_shift = elu(y)+1; then W2
            r1g = grpool.tile([128, GP * HEADS], F32, tag="r1g")
            nc.vector.reciprocal(r1g[:], d1g[:])
            o1v = _view(o1g, 0, [[HW, GP], [HID, HEADS], [1, HID]])
            rev = _view(r1g, 0, [[HEADS, GP], [1, HEADS], [0, HID]])
            nc.vector.tensor_tensor(out=o1v, in0=o1v, in1=rev, op=OP.mult)
            sv = _view(bns_sb, 0, [[0, GP], [1, HW]])
            tv = _view(bnt_sb, 0, [[0, GP], [1, HW]])
            o1f = _view(o1g, 0, [[HW, GP], [1, HW]])
            nc.vector.tensor_tensor(out=o1f, in0=o1f, in1=sv, op=OP.mult)
            nc.vector.tensor_tensor(out=o1f, in0=o1f, in1=tv, op=OP.add)
            mn = scratch.tile([128, GP * HW], F32, tag="mn")
            nc.vector.tensor_scalar_min(mn[:], o1g[:], 0.0)
            nc.scalar.activation(mn[:], mn[:], AF.Exp)
            nc.vector.scalar_tensor_tensor(
                out=o1g[:], in0=o1g[:], scalar=0.0, in1=mn[:],
                op0=OP.max, op1=OP.add,
            )
            for bj in range(GP):
                bb = g * GP + bj
                e2 = small.tile([128, 2 * 128], F32, tag="eluT")
                for j in range(2):
                    pt = pstr.tile([128, 512], F32, tag="tr")
                    nc.tensor.transpose(
                        pt[:, 0:128],
                        o1g[:, bj * HW + j * 128: bj * HW + (j + 1) * 128],
                        ident[:],
                    )
                    nc.vector.tensor_copy(e2[:, j * 128:(j + 1) * 128],
                                          pt[:, 0:128])
                ph2 = psum.tile([128, R2U], F32, tag="ps")
                for j in range(2):
                    nc.tensor.matmul(
                        ph2[:], lhsT=e2[:, j * 128:(j + 1) * 128],
                        rhs=w2e_sb[:, j * R2U:(j + 1) * R2U],
                        start=(j == 0), stop=False,
                    )
                nc.tensor.matmul(ph2[:], lhsT=ones1[:], rhs=nw2s_sb[:],
                                 start=False, stop=True)
                h2b = small.tile([128, HID], BF, tag="h2b")
                nc.vector.tensor_copy(h2b[:], ph2[:, 0:HID])
                h2a = small.tile([128, 2], F32, tag="h2a")
                nc.scalar.copy(h2a[:], ph2[:, HID:HID + 2])
                nc.sync.dma_start(hx2loc[bb * 128:(bb + 1) * 128, 0:HID],
                                  h2b[:])
                nc.sync.dma_start(
                    hx2loc[:].bitcast(F32)[bb * 128:(bb + 1) * 128,
                                           HID // 2:HID // 2 + 2],
                    h2a[:])

        edge_pass(1, hx, R1, R1U, HID, HEADS, HW + HEADS, get_adblk1, post1)

        # ---- AllGather layer-2 table ----
        nc.gpsimd.collective_compute(
            "AllGather", OP.bypass,
            replica_groups=[list(range(cfg.n_cores))],
            ins=[hx2loc[:].opt()],
            outs=[hx2full[:].opt()],
        )

        # ---- Layer 2 + heads ----
        lhsT33 = persist.tile([HID + 1, 128], F32)
        nc.vector.memset(lhsT33[HID:HID + 1, :], 1.0)

        def get_adblk2(b):
            ad2 = adpool.tile([128, 1], F32, tag="ad2")
            nc.sync.dma_start(
                ad2[:],
                hx2loc[:].bitcast(F32)[b * 128:(b + 1) * 128,
                                       HID // 2 + 1:HID // 2 + 2])
            ad2b = adpool.tile([128, 1], BF, tag="ad2b")
            nc.vector.tensor_copy(ad2b[:], ad2[:])
            return ad2b[:]

        def post2(b, ps2):
            den2 = small.tile([128, 1], F32, tag="den2")
            nc.vector.tensor_scalar_add(den2[:], ps2[:, HID:HID + 1], 1e-16)
            rec2 = small.tile([128, 1], F32, tag="rec2")
            nc.vector.reciprocal(rec2[:], den2[:])
            out2 = small.tile([128, HID], F32, tag="out2")
            nc.vector.tensor_scalar(
                out=out2[:], in0=ps2[:, 0:HID], scalar1=rec2[:],
                scalar2=None, op0=OP.mult,
            )
            pt = psum.tile([HID, 128], F32, tag="ps")
            nc.tensor.transpose(pt[:], out2[:], ident[:])
            nc.vector.tensor_copy(lhsT33[0:HID, :], pt[:])
            pz = psum.tile([128, OC], F32, tag="ps")
            nc.tensor.matmul(pz[:], lhsT=lhsT33[:], rhs=wh_sb[:],
                             start=True, stop=True)
            osb = scratch.tile([128, OC], F32, tag="osb")
            e1 = scratch.tile([128, 40], F32, tag="e1")
            se1 = small.tile([128, 1], F32, tag="se")
            nc.scalar.activation(e1[:], pz[:, 0:40], AF.Exp, accum_out=se1[:])
            lse = small.tile([128, 1], F32, tag="lse")
            nc.scalar.activation(lse[:], se1[:], AF.Ln)
            nc.vector.tensor_scalar(
                out=osb[:, 0:40], in0=pz[:, 0:40], scalar1=lse[:],
                scalar2=None, op0=OP.subtract,
            )
            e2h = scratch.tile([128, 40], F32, tag="e2h")
            se2 = small.tile([128, 1], F32, tag="se")
            nc.scalar.activation(e2h[:], pz[:, 40:80], AF.Exp, accum_out=se2[:])
            re2 = small.tile([128, 1], F32, tag="re2")
            nc.vector.reciprocal(re2[:], se2[:])
            nc.vector.tensor_scalar(
                out=osb[:, 40:80], in0=e2h[:], scalar1=re2[:],
                scalar2=None, op0=OP.mult,
            )
            e3 = small.tile([128, 2], F32, tag="e3")
            nc.scalar.activation(e3[:], pz[:, 80:82], AF.Exp, scale=-1.0)
            nc.vector.tensor_scalar_add(e3[:], e3[:], 1.0)
            nc.vector.reciprocal(osb[:, 80:82], e3[:])
            nc.sync.dma_start(out_all[b * 128:(b + 1) * 128, :], osb[:])

        edge_pass(2, hx2full, R2, R2U, HID, 1, HID, get_adblk2, post2)

    nc.compile()
    return nc


# ----------------------------------------------------------------------------
# Host-side graph partitioning / input preparation
# ----------------------------------------------------------------------------
def wrap16(idx_2d):
    """[nblk, n] int -> [128, nblk*(n/16)] int16 (16-wrap, replicated 8x)."""
    nblk, n = idx_2d.shape
    a = idx_2d.reshape(nblk, n // 16, 16).transpose(2, 0, 1).reshape(
        16, nblk * (n // 16))
    return np.ascontiguousarray(np.tile(a.astype(np.int16), (8, 1)))


def partition_graph(edge_index, n_nodes, n_cores, bpc, lo_cap, tlo, thi):
    src = np.concatenate([np.asarray(edge_index[0], dtype=np.int64),
                          np.arange(n_nodes, dtype=np.int64)])
    dst = np.concatenate([np.asarray(edge_index[1], dtype=np.int64),
                          np.arange(n_nodes, dtype=np.int64)])
    n_blocks = n_cores * bpc
    n_slots = n_blocks * 128
    hi_base = max(0, n_slots - lo_cap)
    deg = np.bincount(dst, minlength=n_nodes).astype(np.int64)

    order = np.argsort(-deg, kind="stable")
    loads = np.zeros(n_blocks, dtype=np.int64)
    blk_of_node = np.empty(n_nodes, dtype=np.int64)
    pos = 0
    while pos < n_nodes:
        take = min(n_blocks, n_nodes - pos)
        chunk = order[pos:pos + take]
        light = np.argsort(loads, kind="stable")[:take]
        blk_of_node[chunk] = light
        loads[light] += deg[chunk]
        pos += take

    row_of_node = np.empty(n_nodes, dtype=np.int64)
    blk_sorted = np.argsort(blk_of_node, kind="stable")
    bc = np.bincount(blk_of_node, minlength=n_blocks)
    assert bc.max() <= 128
    starts = np.concatenate([[0], np.cumsum(bc)[:-1]])
    row_of_node[blk_sorted] = np.arange(n_nodes) - starts[blk_of_node[blk_sorted]]
    slot_of_node = blk_of_node * 128 + row_of_node

    eblk = blk_of_node[dst].astype(np.int64)
    esrc_slot = slot_of_node[src]
    edst_row = row_of_node[dst].astype(np.float32)
    is_lo = esrc_slot < lo_cap

    NLO, NHI = tlo * 128, thi * 128
    lo_idx = np.zeros((n_blocks, NLO), dtype=np.int64)
    hi_idx = np.zeros((n_blocks, max(NHI, 1)), dtype=np.int64)
    dstl = np.full((n_blocks, (tlo + thi) * 128), PAD_DSTL, dtype=np.float32)

    # place edges: per block, lo edges then hi edges
    for side, sel in ((0, is_lo), (1, ~is_lo)):
        eb = eblk[sel]
        es = esrc_slot[sel]
        er = edst_row[sel]
        order_e = np.argsort(eb, kind="stable")
        eb, es, er = eb[order_e], es[order_e], er[order_e]
        cnt = np.bincount(eb, minlength=n_blocks)
        st = np.concatenate([[0], np.cumsum(cnt)[:-1]])
        pos_in = np.arange(len(eb)) - st[eb]
        if side == 0:
            assert cnt.max() <= NLO, f"lo overflow {cnt.max()}>{NLO}"
            lo_idx[eb, pos_in] = es
            dstl[eb, pos_in] = er
        else:
            assert cnt.max() <= max(NHI, 1), f"hi overflow {cnt.max()}>{NHI}"
            if NHI:
                hi_idx[eb, pos_in] = es - hi_base
                dstl[eb, NLO + pos_in] = er

    per_core = []
    for k in range(n_cores):
        blo = lo_idx[k * bpc:(k + 1) * bpc]
        pc = {"ilo": wrap16(blo)}
        if NHI:
            pc["ihi"] = wrap16(hi_idx[k * bpc:(k + 1) * bpc])
        gb = (np.arange(bpc) + k * bpc) * 128
        pc["adix"] = np.ascontiguousarray(
            (gb[None, :] + np.arange(128)[:, None]).astype(np.int32))
        d = dstl[k * bpc:(k + 1) * bpc].reshape(bpc, tlo + thi, 128)
        pc["edstl"] = np.ascontiguousarray(
            d.transpose(2, 0, 1).reshape(128, bpc * (tlo + thi))).astype(
                ml_dtypes.bfloat16)
        per_core.append(pc)
    return slot_of_node, per_core


def fold_weights(inp, cfg):
    HID, HEADS, HW = cfg.hid, cfg.heads, cfg.hw
    g = lambda k: np.asarray(inp[k], dtype=np.float64)
    W1, a_src1, a_dst1 = g("W1"), g("a_src1"), g("a_dst1")
    As1 = np.zeros((HW, HEADS))
    Ad1 = np.zeros((HW, HEADS))
    for h in range(HEADS):
        As1[h * HID:(h + 1) * HID, h] = a_src1[h]
        Ad1[h * HID:(h + 1) * HID, h] = a_dst1[h]
    w1ext = np.concatenate([W1, W1 @ As1, W1 @ Ad1], axis=1).astype(np.float32)

    s = g("bn_gamma") / np.sqrt(g("bn_var") + BN_EPS)
    t = (g("b1") - g("bn_mean")) * s + g("bn_beta")
    bns = np.tile(s.astype(np.float32), (128, 1))
    bnt = np.tile(t.astype(np.float32), (128, 1))

    W2 = g("W2")
    w2e = np.concatenate(
        [W2, W2 @ g("a_src2").reshape(-1, 1), W2 @ g("a_dst2").reshape(-1, 1)],
        axis=1)
    negw2sum = (-w2e.sum(axis=0)).astype(np.float32)[None, :]
    w2ext = np.concatenate([w2e[0:128], w2e[128:256]], axis=1).astype(np.float32)

    Wall = np.concatenate([g("Wc"), g("Ws"), g("Wh"), g("We")], axis=1)
    ball = np.concatenate([g("bc"), g("bs"), g("bh"), g("be")])
    ball = ball + g("b2") @ Wall
    wheads = np.concatenate([Wall, ball[None, :]], axis=0).astype(np.float32)
    return w1ext, bns, bnt, w2ext, negw2sum, wheads


def prepare_inputs(inputs, n_cores, bpc, tlo, thi, lo_cap=None):
    x = np.asarray(inputs["x"], dtype=np.float32)
    edge_index = np.asarray(inputs["edge_index"])
    n = x.shape[0]
    cfg = Cfg(n_cores=n_cores, bpc=bpc, tlo=tlo, thi=thi, n_nodes=n,
              lo_cap=lo_cap)
    slot_of_node, per_core = partition_graph(
        edge_index, n, n_cores, bpc, cfg.lo_cap, tlo, thi)

    asg2orig = np.zeros(cfg.n_slots, dtype=np.int64)
    asg2orig[slot_of_node] = np.arange(n)
    xT = np.ascontiguousarray(x[asg2orig].T)

    w1ext, bns, bnt, w2ext, negw2sum, wheads = fold_weights(inputs, cfg)
    in_maps = []
    for k in range(n_cores):
        m = {"xT": xT, "w1ext": w1ext, "w2ext": w2ext, "wheads": wheads,
             "negw2sum": negw2sum, "bns": bns, "bnt": bnt}
        m.update(per_core[k])
        in_maps.append(m)
    return cfg, in_maps, slot_of_node


def assemble_outputs(results, slot_of_node, cfg: Cfg):
    full = np.concatenate(
        [np.asarray(results[k]["out_all"]) for k in range(cfg.n_cores)], axis=0)
    rows = full[slot_of_node]
    return (np.ascontiguousarray(rows[:, 0:40]),
            np.ascontiguousarray(rows[:, 40:80]),
            np.ascontiguousarray(rows[:, 80]),
            np.ascontiguousarray(rows[:, 81]))


# ----------------------------------------------------------------------------
# Entry point
# ----------------------------------------------------------------------------
_compiled = {}


def _get_program(cfg: Cfg):
    key = (cfg.n_cores, cfg.bpc, cfg.tlo, cfg.thi, cfg.n_nodes, cfg.lo_cap)
    if key not in _compiled:
        _compiled[key] = build_program(cfg)
    return _compiled[key]


def kernel(**inputs):
    from concourse import bass_utils

    cfg = in_maps = slot_of_node = None
    for tlo, thi in ((12, 8), (13, 9), (15, 11)):
        try:
            cfg, in_maps, slot_of_node = prepare_inputs(
                inputs, n_cores=8, bpc=50, tlo=tlo, thi=thi)
            break
        except AssertionError:
            continue
    assert cfg is not None, "edge packing failed"
    nc = _get_program(cfg)
    res = bass_utils.run_bass_kernel_spmd(
        nc, in_maps, core_ids=list(range(cfg.n_cores)))
    return assemble_outputs(res.results, slot_of_node, cfg)
